# revision 20
# baseline (speedup 1.0000x reference)
"""AudioVisualSyncAnalyzer Trainium2 kernel (8 NeuronCores, pure data parallel).

v2.1: fp16/f32r DFT pipeline.
  Inputs split hi/lo fp16 (DMA-XBAR transposed); forward 255-pt DFT = 12
  PSUM-accumulated fp16 matmuls per block (wc hi/lo x v hi/lo, lo*lo dropped);
  cross-spectrum on Pool in f32r (sqrt(w)/2-scale folded into inverse coeffs);
  inverse DFT f32r (col 255 = sum trick); stats: Pool max-tree + DVE
  max_index, sumsq/norms via tiny PE matmuls vs ones (partition reduction,
  accumulated into dead PSUM regions); phase coherence via ACT sin; MLP f32r.

Layout per core: 8192 samples = 16 blocks x 512 samples (4 subs x 128
partitions); sample index within block = sub*128 + p.
"""
import sys
import numpy as np

sys.path.insert(0, "/opt/trn_rl_repo")

import concourse.bass as bass
import concourse.bacc as bacc_mod
import concourse.tile as tile
from concourse import mybir
from concourse.bass_utils import run_bass_kernel_spmd

F32 = mybir.dt.float32
F32R = mybir.dt.float32r
FP16 = mybir.dt.float16
U32 = mybir.dt.uint32
AF = mybir.ActivationFunctionType
AX = mybir.AxisListType
ALU = mybir.AluOpType

L = 128          # sequence length
N = 255          # DFT length (odd -> 128 bins)
NB = 128         # bins
B_FULL = 65536
NCORES = 8
SPC = B_FULL // NCORES      # samples per core = 8192
BLK = 512                   # samples per block
NBLK = SPC // BLK           # 16
NSUB = 4
NSUBALL = NBLK * NSUB       # 64
NGRP = NBLK // 2            # 8 dma groups of 1024 samples
PI = float(np.pi)

_CACHED = {}


def _consts():
    t = np.arange(L, dtype=np.float64)[:, None]
    p = np.arange(NB, dtype=np.float64)[None, :]
    ang = 2.0 * np.pi * (t * p) / N
    wc = np.cos(ang)
    ws = np.sin(ang)
    wch = wc.astype(np.float16)
    wcl = (wc - wch.astype(np.float64)).astype(np.float16)
    wsh = ws.astype(np.float16)
    wsl = (ws - wsh.astype(np.float64)).astype(np.float16)
    k = np.arange(N, dtype=np.int64)
    nk = (k - 127) % N
    pp = np.arange(NB, dtype=np.float64)[:, None]
    wtil = np.where(pp == 0, 1.0, 2.0) / N
    sw = np.sqrt(wtil) / 2.0
    angi = 2.0 * np.pi * (pp * nk[None, :]) / N
    ci = np.zeros((NB, 256))
    si = np.zeros((NB, 256))
    ci[:, :N] = (wtil * np.cos(angi)) / sw
    si[:, :N] = (-wtil * np.sin(angi)) / sw
    ci[0, 255] = 1.0 / sw[0, 0]
    return dict(
        wch=wch, wcl=wcl, wsh=wsh, wsl=wsl,
        ci=ci.astype(np.float32), si=si.astype(np.float32),
        sw=sw.astype(np.float32),
        ident=np.eye(128, dtype=np.float32),
        ones=np.ones((128, 1), dtype=np.float16),
    )


def _mlp_consts(W1, b1, W2, b2):
    W1x4 = np.zeros((24, 64), dtype=np.float32)
    W2x4 = np.zeros((64, 128), dtype=np.float32)
    b1x4 = np.zeros((64, 1), dtype=np.float32)
    b2x4 = np.zeros((128, 1), dtype=np.float32)
    for s in range(4):
        W1x4[s * 6:(s + 1) * 6, s * 16:(s + 1) * 16] = W1
        W2x4[s * 16:(s + 1) * 16, s * 32:(s + 1) * 32] = W2
        b1x4[s * 16:(s + 1) * 16, 0] = b1
        b2x4[s * 32:(s + 1) * 32, 0] = b2
    return W1x4, W2x4, b1x4, b2x4


def build_nc():
    nc = bacc_mod.Bacc("TRN2", target_bir_lowering=False)
    v_in = nc.declare_dram_parameter("video", [SPC, L], F32, isOutput=False)
    a_in = nc.declare_dram_parameter("audio", [SPC, L], F32, isOutput=False)
    wch_in = nc.declare_dram_parameter("wch", [L, NB], FP16, isOutput=False)
    wcl_in = nc.declare_dram_parameter("wcl", [L, NB], FP16, isOutput=False)
    wsh_in = nc.declare_dram_parameter("wsh", [L, NB], FP16, isOutput=False)
    wsl_in = nc.declare_dram_parameter("wsl", [L, NB], FP16, isOutput=False)
    ci_in = nc.declare_dram_parameter("ci", [NB, 256], F32, isOutput=False)
    si_in = nc.declare_dram_parameter("si", [NB, 256], F32, isOutput=False)
    sw_in = nc.declare_dram_parameter("sw", [NB, 1], F32, isOutput=False)
    id_in = nc.declare_dram_parameter("ident", [128, 128], F32, isOutput=False)
    on_in = nc.declare_dram_parameter("ones", [128, 1], FP16, isOutput=False)
    w1_in = nc.declare_dram_parameter("w1x4", [24, 64], F32, isOutput=False)
    w2_in = nc.declare_dram_parameter("w2x4", [64, 128], F32, isOutput=False)
    b1_in = nc.declare_dram_parameter("b1x4", [64, 1], F32, isOutput=False)
    b2_in = nc.declare_dram_parameter("b2x4", [128, 1], F32, isOutput=False)
    out = nc.declare_dram_parameter("out", [SPC, 32], F32, isOutput=True)

    with tile.TileContext(nc) as tc:
        with tc.tile_pool(name="singles", bufs=1) as singles, \
             tc.tile_pool(name="stats", bufs=1) as stats:
            wch = singles.tile([L, NB], FP16, tag="wch")
            wcl = singles.tile([L, NB], FP16, tag="wcl")
            wsh = singles.tile([L, NB], FP16, tag="wsh")
            wsl = singles.tile([L, NB], FP16, tag="wsl")
            cif = singles.tile([NB, 256], F32, tag="cif")
            sif = singles.tile([NB, 256], F32, tag="sif")
            ci = singles.tile([NB, 256], F32R, tag="ci")
            si = singles.tile([NB, 256], F32R, tag="si")
            sw = singles.tile([NB, 1], F32, tag="sw")
            ident = singles.tile([128, 128], F32, tag="ident")
            identr = singles.tile([128, 128], F32R, tag="identr")
            ones = singles.tile([128, 1], FP16, tag="ones")
            w1f = singles.tile([24, 64], F32, tag="w1f")
            w2f = singles.tile([64, 128], F32, tag="w2f")
            w1 = singles.tile([24, 64], F32R, tag="w1")
            w2 = singles.tile([64, 128], F32R, tag="w2")
            b1 = singles.tile([64, 1], F32, tag="b1")
            b2 = singles.tile([128, 1], F32, tag="b2")
            zero_b = singles.tile([128, 1], F32, tag="zero_b")
            halfpi_b = singles.tile([128, 1], F32, tag="halfpi_b")
            for t_, h_ in ((wch, wch_in), (wcl, wcl_in), (wsh, wsh_in), (wsl, wsl_in),
                           (cif, ci_in), (sif, si_in), (sw, sw_in), (ident, id_in),
                           (ones, on_in), (w1f, w1_in), (w2f, w2_in),
                           (b1, b1_in), (b2, b2_in)):
                nc.sync.dma_start(out=t_, in_=h_[:])
            nc.vector.tensor_copy(out=ci, in_=cif)
            nc.vector.tensor_copy(out=si, in_=sif)
            nc.vector.tensor_copy(out=identr, in_=ident)
            nc.vector.tensor_copy(out=w1, in_=w1f)
            nc.vector.tensor_copy(out=w2, in_=w2f)
            nc.scalar.mul(out=zero_b, in_=ident[:, 0:1], mul=0.0)
            nc.scalar.activation(out=halfpi_b, in_=zero_b, func=AF.Copy,
                                 bias=PI / 2, scale=0.0)

            # staged fp16 data for all blocks (dim1 = signal: 0=video 1=audio)
            SH = stats.tile([128, NGRP, 2, 8, L], FP16, tag="SH")  # sample-major hi
            TH = stats.tile([128, NGRP, 2, 8, L], FP16, tag="TH")  # time-major hi
            TL = stats.tile([128, NGRP, 2, 8, L], FP16, tag="TL")  # time-major lo
            nsq = stats.tile([128, 2, NSUBALL], F32, tag="nsq")
            inv_v = stats.tile([128, NSUBALL], F32, tag="inv_v")
            inv_a = stats.tile([128, NSUBALL], F32, tag="inv_a")
            cmax = stats.tile([128, NBLK, 4], F32, tag="cmax")
            idxf = stats.tile([128, NBLK, 4], F32, tag="idxf")
            s_c = stats.tile([128, NBLK, 4], F32, tag="s_c")
            ssq_c = stats.tile([128, NBLK, 4], F32, tag="ssq_c")
            mc = stats.tile([128, NBLK, 4], F32, tag="mc")
            ms = stats.tile([128, NBLK, 4], F32, tag="ms")
            s_all = stats.tile([128, NBLK, 4, 6], F32R, tag="s_all")
            im8 = stats.tile([128, 8], F32, tag="im8")
            nc.vector.memset(im8[:, 4:8], -3.0e38)

            with tc.tile_pool(name="inp", bufs=3) as inp, \
                 tc.tile_pool(name="sqp", bufs=2) as sqp, \
                 tc.tile_pool(name="pwork", bufs=2) as pwork, \
                 tc.tile_pool(name="mxw", bufs=2) as mxw, \
                 tc.tile_pool(name="psf", bufs=1, space="PSUM") as psf, \
                 tc.tile_pool(name="psc", bufs=2, space="PSUM") as psc:

                vaf_tiles = {}

                def phase0_dma(g):
                    r0 = g * 1024
                    vaf = inp.tile([128, 2, 8, L], F32, tag="vaf")
                    vaf_tiles[g] = vaf
                    nc.sync.dma_start(out=vaf[:, 0], in_=v_in[r0:r0 + 1024, :].rearrange(
                        "(sub p) t -> p sub t", p=128))
                    nc.sync.dma_start(out=vaf[:, 1], in_=a_in[r0:r0 + 1024, :].rearrange(
                        "(sub p) t -> p sub t", p=128))

                def phase0(g):
                    c0 = g * 8
                    vaf = vaf_tiles.pop(g)
                    shs = SH[:, g]
                    nc.vector.tensor_copy(out=shs, in_=vaf)        # round hi
                    lb = inp.tile([128, 2, 8, L], FP16, tag="lb")  # residual lo
                    nc.vector.tensor_sub(lb[:, 0], vaf[:, 0], shs[:, 0])
                    nc.gpsimd.tensor_sub(lb[:, 1], vaf[:, 1], shs[:, 1])
                    nc.sync.dma_start_transpose(TH[:, g],
                                                shs.rearrange("p a b c -> p (a b c)"))
                    nc.sync.dma_start_transpose(TL[:, g],
                                                lb.rearrange("p a b c -> p (a b c)"))

                def phase1(blk, norm_g):
                    gb, lo4 = blk // 2, (blk % 2) * 4
                    vThb = TH[:, gb, 0, lo4:lo4 + 4, :].rearrange("p a b -> p (a b)")
                    aThb = TH[:, gb, 1, lo4:lo4 + 4, :].rearrange("p a b -> p (a b)")
                    vTlb = TL[:, gb, 0, lo4:lo4 + 4, :].rearrange("p a b -> p (a b)")
                    aTlb = TL[:, gb, 1, lo4:lo4 + 4, :].rearrange("p a b -> p (a b)")
                    vc_ps = psf.tile([NB, BLK], F32, tag="vc_ps")
                    vs_ps = psf.tile([NB, BLK], F32, tag="vs_ps")
                    ac_ps = psf.tile([NB, BLK], F32, tag="ac_ps")
                    as_ps = psf.tile([NB, BLK], F32, tag="as_ps")
                    for dst, w_h, w_l, r_h, r_l in (
                            (vc_ps, wch, wcl, vThb, vTlb),
                            (vs_ps, wsh, wsl, vThb, vTlb),
                            (ac_ps, wch, wcl, aThb, aTlb),
                            (as_ps, wsh, wsl, aThb, aTlb)):
                        nc.tensor.matmul(dst, w_h, r_h, start=True, stop=False)
                        nc.tensor.matmul(dst, w_h, r_l, start=False, stop=False)
                        nc.tensor.matmul(dst, w_l, r_h, start=False, stop=True)
                    # exit the forward outputs (gpsimd cannot touch PSUM);
                    # the copy also applies the sqrt(w)/2 scale on the v side
                    vcx = pwork.tile([NB, BLK], F32R, tag="vcx")
                    vsx = pwork.tile([NB, BLK], F32R, tag="vsx")
                    acx = pwork.tile([NB, BLK], F32R, tag="acx")
                    asx = pwork.tile([NB, BLK], F32R, tag="asx")
                    nc.scalar.mul(out=vcx, in_=vc_ps, mul=sw[:, 0:1])
                    nc.scalar.mul(out=vsx, in_=vs_ps, mul=sw[:, 0:1])
                    nc.scalar.copy(out=acx, in_=ac_ps)
                    nc.scalar.copy(out=asx, in_=as_ps)
                    t1 = pwork.tile([NB, BLK], F32R, tag="t1")
                    t2 = pwork.tile([NB, BLK], F32R, tag="t2")
                    t3 = pwork.tile([NB, BLK], F32R, tag="t3")
                    t4 = pwork.tile([NB, BLK], F32R, tag="t4")
                    nc.gpsimd.tensor_mul(t1, vcx, acx)
                    nc.gpsimd.tensor_mul(t2, vsx, asx)
                    nc.gpsimd.tensor_mul(t3, vcx, asx)
                    nc.gpsimd.tensor_mul(t4, vsx, acx)
                    pr = pwork.tile([NB, BLK], F32R, tag="pr")
                    pi = pwork.tile([NB, BLK], F32R, tag="pi")
                    nc.gpsimd.tensor_add(pr, t1, t2)
                    nc.gpsimd.tensor_sub(pi, t3, t4)
                    corr = psc.tile([128, NSUB, 256], F32, tag="corr")
                    for s in range(NSUB):
                        sl = slice(s * 128, (s + 1) * 128)
                        nc.tensor.matmul(corr[:, s, :], pr[:, sl], ci, start=True, stop=False)
                        nc.tensor.matmul(corr[:, s, :], pi[:, sl], si, start=False, stop=True)
                    # sumsq: sum_k corr^2 = 4*sum_p(pr^2+pi^2) via ACT squares + tiny mm
                    sq1 = pwork.tile([NB, BLK], FP16, tag="sq1")
                    sq2 = pwork.tile([NB, BLK], FP16, tag="sq2")
                    nc.gpsimd.tensor_mul(sq1, pr, pr)
                    nc.gpsimd.tensor_mul(sq2, pi, pi)
                    ssq_ps = vc_ps[:, 8:12]          # dead region after muls
                    for s in range(NSUB):
                        sl = slice(s * 128, (s + 1) * 128)
                        nc.tensor.matmul(ssq_ps[:, s:s + 1], sq1[:, sl], ones,
                                         start=True, stop=False)
                        nc.tensor.matmul(ssq_ps[:, s:s + 1], sq2[:, sl], ones,
                                         start=False, stop=True)
                    nc.vector.tensor_copy(out=ssq_c[:, blk, :], in_=ssq_ps)
                    nc.vector.reduce_max(im8[:, 0:4], corr[:, :, 0:255], axis=AX.X)
                    nc.vector.tensor_copy(out=cmax[:, blk, :], in_=im8[:, 0:4])
                    idx8 = mxw.tile([128, 8], U32, tag="idx8")
                    nc.vector.max_index(idx8, im8, corr.rearrange("p a b -> p (a b)"))
                    nc.vector.tensor_copy(out=idxf[:, blk, :], in_=idx8[:, 0:4])
                    nc.vector.tensor_copy(out=s_c[:, blk, :], in_=corr[:, :, 255])
                    # norm tiny-mms for group `norm_g` land in this block's
                    # as_ps[:, 8:24] (dead after the stt muls consumed it)
                    if norm_g is not None:
                        cp = norm_g * 8
                        vsq = sqp.tile([128, 8, L], FP16, tag="vsq")
                        asq = sqp.tile([128, 8, L], FP16, tag="asq")
                        nc.gpsimd.tensor_mul(vsq, TH[:, norm_g, 0], TH[:, norm_g, 0])
                        nc.gpsimd.tensor_mul(asq, TH[:, norm_g, 1], TH[:, norm_g, 1])
                        for s in range(8):
                            nc.tensor.matmul(as_ps[:, 8 + s:9 + s], vsq[:, s, :], ones,
                                             start=True, stop=True)
                            nc.tensor.matmul(as_ps[:, 16 + s:17 + s], asq[:, s, :], ones,
                                             start=True, stop=True)
                        nc.vector.tensor_copy(out=nsq[:, 0, cp:cp + 8], in_=as_ps[:, 8:16])
                        nc.vector.tensor_copy(out=nsq[:, 1, cp:cp + 8], in_=as_ps[:, 16:24])

                nall = stats.tile([128, 2, NSUBALL], F32, tag="nall")

                def inv_part(lo, hi):
                    nc.vector.tensor_scalar_max(nsq[:, :, lo:hi], nsq[:, :, lo:hi], 1e-24)
                    nc.scalar.activation(out=nall[:, :, lo:hi], in_=nsq[:, :, lo:hi],
                                         func=AF.Sqrt, bias=zero_b)
                    nc.vector.reciprocal(out=inv_v[:, lo:hi], in_=nall[:, 0, lo:hi])
                    nc.vector.reciprocal(out=inv_a[:, lo:hi], in_=nall[:, 1, lo:hi])

                def sins_blk(blk):
                    c0 = blk * 4
                    a_sc = sqp.tile([128, NSUB, L], FP16, tag="a_sc")
                    v_sc = sqp.tile([128, NSUB, L], FP16, tag="v_sc")
                    d_t = sqp.tile([128, NSUB, L], FP16, tag="d_t")
                    for s in range(NSUB):
                        gs, ls = (c0 + s) // 8, (c0 + s) % 8
                        nc.gpsimd.tensor_scalar_mul(a_sc[:, s, :], SH[:, gs, 1, ls, :],
                                                    inv_a[:, c0 + s:c0 + s + 1])
                        nc.gpsimd.tensor_scalar_mul(v_sc[:, s, :], SH[:, gs, 0, ls, :],
                                                    inv_v[:, c0 + s:c0 + s + 1])
                    nc.gpsimd.tensor_sub(d_t, v_sc, a_sc)
                    sj = sqp.tile([128, NSUB, L], FP16, tag="sj")
                    cj = sqp.tile([128, NSUB, L], FP16, tag="cj")
                    nc.scalar.activation(out=sj, in_=d_t, func=AF.Sin, bias=zero_b)
                    nc.scalar.activation(out=cj, in_=d_t, func=AF.Sin, bias=halfpi_b)
                    nc.vector.reduce_sum(ms[:, blk, :], sj, axis=AX.X)
                    nc.vector.reduce_sum(mc[:, blk, :], cj, axis=AX.X)

                # interleaved emission: dma prefetch, phase0(g), blocks of
                # g-1, early sins once the first inv half is ready
                phase0_dma(0)
                sins_sched = {5: [0, 1, 2], 6: [3, 4, 5], 7: [6, 7]}
                for g in range(NGRP):
                    if g + 1 < NGRP:
                        phase0_dma(g + 1)
                    phase0(g)
                    if g >= 1:
                        phase1(2 * (g - 1), None)
                        phase1(2 * (g - 1) + 1, g - 1)
                    if g == 4:
                        inv_part(0, 32)
                    for b in sins_sched.get(g, []):
                        sins_blk(b)
                phase1(NBLK - 2, None)
                phase1(NBLK - 1, NGRP - 1)
                inv_part(32, 64)
                for b in range(8, NBLK):
                    sins_blk(b)

            # phase 2: assemble features + MLP
            with tc.tile_pool(name="asm", bufs=1) as asm, \
                 tc.tile_pool(name="mlpw", bufs=4) as mlpw, \
                 tc.tile_pool(name="psm", bufs=2, space="PSUM") as psm:
                sh = [128, NBLK, 4]
                u = asm.tile(sh, F32, tag="u")
                u2 = asm.tile(sh, F32, tag="u2")
                tA = asm.tile(sh, F32, tag="tA")
                tB = asm.tile(sh, F32, tag="tB")
                tC = asm.tile(sh, F32, tag="tC")
                iv4 = inv_v.rearrange("p (a b) -> p a b", b=4)
                ia4 = inv_a.rearrange("p (a b) -> p a b", b=4)
                nc.vector.tensor_mul(u, iv4, ia4)
                nc.vector.tensor_mul(u2, u, u)
                uq = asm.tile(sh, F32, tag="uq")
                nc.vector.tensor_scalar_mul(uq, u, 1.0 / (1.0 + 1e-6))
                nc.vector.tensor_mul(s_all[:, :, :, 1], cmax, uq)
                nc.vector.tensor_scalar_mul(tA, s_c, 1.0 / 255.0)
                nc.vector.tensor_mul(tA, tA, u)
                nc.vector.tensor_mul(tB, tA, tA)
                nc.vector.tensor_scalar_mul(tC, ssq_c, 4.0 / 255.0)
                nc.vector.tensor_mul(tC, tC, u2)
                nc.vector.tensor_sub(tC, tC, tB)
                nc.vector.tensor_scalar_max(tC, tC, 0.0)
                nc.scalar.activation(out=s_all[:, :, :, 2], in_=tC, func=AF.Sqrt, bias=zero_b)
                nc.vector.tensor_scalar(out=s_all[:, :, :, 3], in0=u, scalar1=0.0,
                                        scalar2=1.0 / (1.0 + 1e-6), op0=ALU.mult,
                                        op1=ALU.add)
                for s in range(4):
                    off = float(s * 256 + 127)
                    nc.vector.tensor_scalar(out=tA[:, :, s], in0=idxf[:, :, s],
                                            scalar1=off, scalar2=0.1,
                                            op0=ALU.subtract, op1=ALU.mult)
                nc.vector.tensor_copy(out=s_all[:, :, :, 0], in_=tA)
                nc.scalar.activation(out=tB, in_=tA, func=AF.Abs, bias=zero_b, scale=10.0)
                nc.vector.tensor_scalar(out=tB, in0=tB, scalar1=1.0, scalar2=None,
                                        op0=ALU.add)
                nc.vector.reciprocal(out=tC, in_=tB)
                nc.vector.tensor_copy(out=s_all[:, :, :, 5], in_=tC)
                nc.vector.tensor_mul(tA, mc, mc)
                nc.vector.tensor_mul(tB, ms, ms)
                nc.vector.tensor_add(tC, tA, tB)
                nc.scalar.activation(out=s_all[:, :, :, 4], in_=tC, func=AF.Sqrt,
                                     bias=zero_b, scale=1.0 / (128.0 * 128.0))

                for g in range(NBLK // 4):
                    sT_ps = psm.tile([24, BLK], F32R, tag="sT_ps")
                    for t_ in range(4):
                        b = g * 4 + t_
                        nc.tensor.transpose(sT_ps[:, t_ * 128:(t_ + 1) * 128],
                                            s_all[:, b, :, :].rearrange("p a b -> p (a b)"),
                                            identr)
                    sT = mlpw.tile([24, BLK], F32R, tag="sT")
                    nc.scalar.copy(out=sT, in_=sT_ps)
                    h_ps = psm.tile([64, BLK], F32, tag="h_ps")
                    nc.tensor.matmul(h_ps, w1, sT, start=True, stop=True)
                    h_sb = mlpw.tile([64, BLK], F32R, tag="h_sb")
                    nc.scalar.activation(out=h_sb, in_=h_ps, func=AF.Relu, bias=b1)
                    o_ps = psm.tile([128, BLK], F32, tag="o_ps")
                    nc.tensor.matmul(o_ps, w2, h_sb, start=True, stop=True)
                    o_sb = mlpw.tile([128, BLK], F32R, tag="o_sb")
                    nc.scalar.activation(out=o_sb, in_=o_ps, func=AF.Identity, bias=b2)
                    oT_ps = psm.tile([128, BLK], F32R, tag="oT_ps")
                    for t_ in range(4):
                        nc.tensor.transpose(oT_ps[:, t_ * 128:(t_ + 1) * 128],
                                            o_sb[:, t_ * 128:(t_ + 1) * 128], identr)
                    oT = mlpw.tile([128, BLK], F32, tag="oT")
                    nc.vector.tensor_copy(out=oT, in_=oT_ps)
                    nc.sync.dma_start(
                        out=out[g * 4 * BLK:(g + 1) * 4 * BLK, :].rearrange(
                            "(blk sub p) f -> p blk sub f", p=128, sub=4),
                        in_=oT.rearrange("p (blk sub f) -> p blk sub f", sub=4, f=32))
    nc.compile()
    return nc


def make_in_map(video, audio, W1, b1, W2, b2):
    c = _consts()
    W1x4, W2x4, b1x4, b2x4 = _mlp_consts(
        np.asarray(W1, np.float32), np.asarray(b1, np.float32),
        np.asarray(W2, np.float32), np.asarray(b2, np.float32))
    return {
        "video": video, "audio": audio,
        "wch": c["wch"], "wcl": c["wcl"], "wsh": c["wsh"], "wsl": c["wsl"],
        "ci": c["ci"], "si": c["si"], "sw": c["sw"], "ident": c["ident"],
        "ones": c["ones"],
        "w1x4": W1x4, "w2x4": W2x4, "b1x4": b1x4, "b2x4": b2x4,
    }


def kernel(video_features, audio_features, W1, b1, W2, b2):
    video_features = np.ascontiguousarray(np.asarray(video_features, dtype=np.float32))
    audio_features = np.ascontiguousarray(np.asarray(audio_features, dtype=np.float32))
    if "nc" not in _CACHED:
        _CACHED["nc"] = build_nc()
    nc = _CACHED["nc"]

    in_maps = []
    for i in range(NCORES):
        sl = slice(i * SPC, (i + 1) * SPC)
        in_maps.append(make_in_map(video_features[sl], audio_features[sl],
                                   W1, b1, W2, b2))
    res = run_bass_kernel_spmd(nc, in_maps, list(range(NCORES)))
    out = np.concatenate([res.results[i]["out"] for i in range(NCORES)], axis=0)
    return out.astype(np.float32)


if __name__ == "__main__":
    rng = np.random.default_rng(0)
    inputs = dict(
        video_features=rng.standard_normal((B_FULL, L)).astype(np.float32),
        audio_features=rng.standard_normal((B_FULL, L)).astype(np.float32),
        W1=(rng.standard_normal((6, 16)) * 0.3).astype(np.float32),
        b1=(rng.standard_normal((16,)) * 0.1).astype(np.float32),
        W2=(rng.standard_normal((16, 32)) * 0.2).astype(np.float32),
        b2=(rng.standard_normal((32,)) * 0.1).astype(np.float32),
    )
    out = kernel(**inputs)
    print("out", out.shape, out.dtype, np.abs(out).mean())


# revision 21
# speedup vs baseline: 1.0034x; 1.0034x over previous
"""AudioVisualSyncAnalyzer Trainium2 kernel (8 NeuronCores, pure data parallel).

v2.1: fp16/f32r DFT pipeline.
  Inputs split hi/lo fp16 (DMA-XBAR transposed); forward 255-pt DFT = 12
  PSUM-accumulated fp16 matmuls per block (wc hi/lo x v hi/lo, lo*lo dropped);
  cross-spectrum on Pool in f32r (sqrt(w)/2-scale folded into inverse coeffs);
  inverse DFT f32r (col 255 = sum trick); stats: Pool max-tree + DVE
  max_index, sumsq/norms via tiny PE matmuls vs ones (partition reduction,
  accumulated into dead PSUM regions); phase coherence via ACT sin; MLP f32r.

Layout per core: 8192 samples = 16 blocks x 512 samples (4 subs x 128
partitions); sample index within block = sub*128 + p.
"""
import sys
import numpy as np

sys.path.insert(0, "/opt/trn_rl_repo")

import concourse.bass as bass
import concourse.bacc as bacc_mod
import concourse.tile as tile
from concourse import mybir
from concourse.bass_utils import run_bass_kernel_spmd

F32 = mybir.dt.float32
F32R = mybir.dt.float32r
FP16 = mybir.dt.float16
U32 = mybir.dt.uint32
AF = mybir.ActivationFunctionType
AX = mybir.AxisListType
ALU = mybir.AluOpType

L = 128          # sequence length
N = 255          # DFT length (odd -> 128 bins)
NB = 128         # bins
B_FULL = 65536
NCORES = 8
SPC = B_FULL // NCORES      # samples per core = 8192
BLK = 512                   # samples per block
NBLK = SPC // BLK           # 16
NSUB = 4
NSUBALL = NBLK * NSUB       # 64
NGRP = NBLK // 2            # 8 dma groups of 1024 samples
PI = float(np.pi)

_CACHED = {}


def _consts():
    t = np.arange(L, dtype=np.float64)[:, None]
    p = np.arange(NB, dtype=np.float64)[None, :]
    ang = 2.0 * np.pi * (t * p) / N
    wc = np.cos(ang)
    ws = np.sin(ang)
    wch = wc.astype(np.float16)
    wcl = (wc - wch.astype(np.float64)).astype(np.float16)
    wsh = ws.astype(np.float16)
    wsl = (ws - wsh.astype(np.float64)).astype(np.float16)
    k = np.arange(N, dtype=np.int64)
    nk = (k - 127) % N
    pp = np.arange(NB, dtype=np.float64)[:, None]
    wtil = np.where(pp == 0, 1.0, 2.0) / N
    sw = np.sqrt(wtil) / 2.0
    angi = 2.0 * np.pi * (pp * nk[None, :]) / N
    ci = np.zeros((NB, 256))
    si = np.zeros((NB, 256))
    ci[:, :N] = (wtil * np.cos(angi)) / sw
    si[:, :N] = (-wtil * np.sin(angi)) / sw
    ci[0, 255] = 1.0 / sw[0, 0]
    return dict(
        wch=wch, wcl=wcl, wsh=wsh, wsl=wsl,
        ci=ci.astype(np.float32), si=si.astype(np.float32),
        sw=sw.astype(np.float32),
        ident=np.eye(128, dtype=np.float32),
        ones=np.ones((128, 1), dtype=np.float16),
    )


def _mlp_consts(W1, b1, W2, b2):
    W1x4 = np.zeros((24, 64), dtype=np.float32)
    W2x4 = np.zeros((64, 128), dtype=np.float32)
    b1x4 = np.zeros((64, 1), dtype=np.float32)
    b2x4 = np.zeros((128, 1), dtype=np.float32)
    for s in range(4):
        W1x4[s * 6:(s + 1) * 6, s * 16:(s + 1) * 16] = W1
        W2x4[s * 16:(s + 1) * 16, s * 32:(s + 1) * 32] = W2
        b1x4[s * 16:(s + 1) * 16, 0] = b1
        b2x4[s * 32:(s + 1) * 32, 0] = b2
    return W1x4, W2x4, b1x4, b2x4


def build_nc():
    nc = bacc_mod.Bacc("TRN2", target_bir_lowering=False)
    v_in = nc.declare_dram_parameter("video", [SPC, L], F32, isOutput=False)
    a_in = nc.declare_dram_parameter("audio", [SPC, L], F32, isOutput=False)
    wch_in = nc.declare_dram_parameter("wch", [L, NB], FP16, isOutput=False)
    wcl_in = nc.declare_dram_parameter("wcl", [L, NB], FP16, isOutput=False)
    wsh_in = nc.declare_dram_parameter("wsh", [L, NB], FP16, isOutput=False)
    wsl_in = nc.declare_dram_parameter("wsl", [L, NB], FP16, isOutput=False)
    ci_in = nc.declare_dram_parameter("ci", [NB, 256], F32, isOutput=False)
    si_in = nc.declare_dram_parameter("si", [NB, 256], F32, isOutput=False)
    sw_in = nc.declare_dram_parameter("sw", [NB, 1], F32, isOutput=False)
    id_in = nc.declare_dram_parameter("ident", [128, 128], F32, isOutput=False)
    on_in = nc.declare_dram_parameter("ones", [128, 1], FP16, isOutput=False)
    w1_in = nc.declare_dram_parameter("w1x4", [24, 64], F32, isOutput=False)
    w2_in = nc.declare_dram_parameter("w2x4", [64, 128], F32, isOutput=False)
    b1_in = nc.declare_dram_parameter("b1x4", [64, 1], F32, isOutput=False)
    b2_in = nc.declare_dram_parameter("b2x4", [128, 1], F32, isOutput=False)
    out = nc.declare_dram_parameter("out", [SPC, 32], F32, isOutput=True)

    with tile.TileContext(nc) as tc:
        with tc.tile_pool(name="singles", bufs=1) as singles, \
             tc.tile_pool(name="stats", bufs=1) as stats:
            wch = singles.tile([L, NB], FP16, tag="wch")
            wcl = singles.tile([L, NB], FP16, tag="wcl")
            wsh = singles.tile([L, NB], FP16, tag="wsh")
            wsl = singles.tile([L, NB], FP16, tag="wsl")
            cif = singles.tile([NB, 256], F32, tag="cif")
            sif = singles.tile([NB, 256], F32, tag="sif")
            ci = singles.tile([NB, 256], F32R, tag="ci")
            si = singles.tile([NB, 256], F32R, tag="si")
            sw = singles.tile([NB, 1], F32, tag="sw")
            ident = singles.tile([128, 128], F32, tag="ident")
            identr = singles.tile([128, 128], F32R, tag="identr")
            ones = singles.tile([128, 1], FP16, tag="ones")
            w1f = singles.tile([24, 64], F32, tag="w1f")
            w2f = singles.tile([64, 128], F32, tag="w2f")
            w1 = singles.tile([24, 64], F32R, tag="w1")
            w2 = singles.tile([64, 128], F32R, tag="w2")
            b1 = singles.tile([64, 1], F32, tag="b1")
            b2 = singles.tile([128, 1], F32, tag="b2")
            zero_b = singles.tile([128, 1], F32, tag="zero_b")
            halfpi_b = singles.tile([128, 1], F32, tag="halfpi_b")
            for t_, h_ in ((wch, wch_in), (wcl, wcl_in), (wsh, wsh_in), (wsl, wsl_in),
                           (cif, ci_in), (sif, si_in), (sw, sw_in), (ident, id_in),
                           (ones, on_in), (w1f, w1_in), (w2f, w2_in),
                           (b1, b1_in), (b2, b2_in)):
                nc.sync.dma_start(out=t_, in_=h_[:])
            nc.vector.tensor_copy(out=ci, in_=cif)
            nc.vector.tensor_copy(out=si, in_=sif)
            nc.vector.tensor_copy(out=identr, in_=ident)
            nc.vector.tensor_copy(out=w1, in_=w1f)
            nc.vector.tensor_copy(out=w2, in_=w2f)
            nc.scalar.mul(out=zero_b, in_=ident[:, 0:1], mul=0.0)
            nc.scalar.activation(out=halfpi_b, in_=zero_b, func=AF.Copy,
                                 bias=PI / 2, scale=0.0)

            # staged fp16 data for all blocks (dim1 = signal: 0=video 1=audio)
            SH = stats.tile([128, NGRP, 2, 8, L], FP16, tag="SH")  # sample-major hi
            TH = stats.tile([128, NGRP, 2, 8, L], FP16, tag="TH")  # time-major hi
            TL = stats.tile([128, NGRP, 2, 8, L], FP16, tag="TL")  # time-major lo
            nsq = stats.tile([128, 2, NSUBALL], F32, tag="nsq")
            inv_v = stats.tile([128, NSUBALL], F32, tag="inv_v")
            inv_a = stats.tile([128, NSUBALL], F32, tag="inv_a")
            cmax = stats.tile([128, NBLK, 4], F32, tag="cmax")
            idxf = stats.tile([128, NBLK, 4], F32, tag="idxf")
            s_c = stats.tile([128, NBLK, 4], F32, tag="s_c")
            ssq_c = stats.tile([128, NBLK, 4], F32, tag="ssq_c")
            mc = stats.tile([128, NBLK, 4], F32, tag="mc")
            ms = stats.tile([128, NBLK, 4], F32, tag="ms")
            s_all = stats.tile([128, NBLK, 4, 6], F32R, tag="s_all")
            im8 = stats.tile([128, 8], F32, tag="im8")
            nc.vector.memset(im8[:, 4:8], -3.0e38)

            with tc.tile_pool(name="inp", bufs=3) as inp, \
                 tc.tile_pool(name="sqp", bufs=2) as sqp, \
                 tc.tile_pool(name="pwork", bufs=2) as pwork, \
                 tc.tile_pool(name="mxw", bufs=2) as mxw, \
                 tc.tile_pool(name="psf", bufs=1, space="PSUM") as psf, \
                 tc.tile_pool(name="psc", bufs=2, space="PSUM") as psc:

                vaf_tiles = {}

                def phase0_dma(g):
                    r0 = g * 1024
                    vaf = inp.tile([128, 2, 8, L], F32, tag="vaf")
                    vaf_tiles[g] = vaf
                    nc.sync.dma_start(out=vaf[:, 0], in_=v_in[r0:r0 + 1024, :].rearrange(
                        "(sub p) t -> p sub t", p=128))
                    nc.sync.dma_start(out=vaf[:, 1], in_=a_in[r0:r0 + 1024, :].rearrange(
                        "(sub p) t -> p sub t", p=128))

                def phase0(g):
                    c0 = g * 8
                    vaf = vaf_tiles.pop(g)
                    shs = SH[:, g]
                    nc.vector.tensor_copy(out=shs, in_=vaf)        # round hi
                    lb = inp.tile([128, 2, 8, L], FP16, tag="lb")  # residual lo
                    nc.vector.tensor_sub(lb[:, 0], vaf[:, 0], shs[:, 0])
                    nc.gpsimd.tensor_sub(lb[:, 1], vaf[:, 1], shs[:, 1])
                    nc.sync.dma_start_transpose(TH[:, g],
                                                shs.rearrange("p a b c -> p (a b c)"))
                    nc.sync.dma_start_transpose(TL[:, g],
                                                lb.rearrange("p a b c -> p (a b c)"))

                def phase1(blk, norm_g):
                    gb, lo4 = blk // 2, (blk % 2) * 4
                    vThb = TH[:, gb, 0, lo4:lo4 + 4, :].rearrange("p a b -> p (a b)")
                    aThb = TH[:, gb, 1, lo4:lo4 + 4, :].rearrange("p a b -> p (a b)")
                    vTlb = TL[:, gb, 0, lo4:lo4 + 4, :].rearrange("p a b -> p (a b)")
                    aTlb = TL[:, gb, 1, lo4:lo4 + 4, :].rearrange("p a b -> p (a b)")
                    vc_ps = psf.tile([NB, BLK], F32, tag="vc_ps")
                    vs_ps = psf.tile([NB, BLK], F32, tag="vs_ps")
                    ac_ps = psf.tile([NB, BLK], F32, tag="ac_ps")
                    as_ps = psf.tile([NB, BLK], F32, tag="as_ps")
                    for dst, w_h, w_l, r_h, r_l in (
                            (vc_ps, wch, wcl, vThb, vTlb),
                            (vs_ps, wsh, wsl, vThb, vTlb),
                            (ac_ps, wch, wcl, aThb, aTlb),
                            (as_ps, wsh, wsl, aThb, aTlb)):
                        nc.tensor.matmul(dst, w_h, r_h, start=True, stop=False)
                        nc.tensor.matmul(dst, w_h, r_l, start=False, stop=False)
                        nc.tensor.matmul(dst, w_l, r_h, start=False, stop=True)
                    # exit the forward outputs (gpsimd cannot touch PSUM);
                    # the copy also applies the sqrt(w)/2 scale on the v side
                    vcx = pwork.tile([NB, BLK], F32R, tag="vcx")
                    vsx = pwork.tile([NB, BLK], F32R, tag="vsx")
                    acx = pwork.tile([NB, BLK], F32R, tag="acx")
                    asx = pwork.tile([NB, BLK], F32R, tag="asx")
                    nc.scalar.mul(out=vcx, in_=vc_ps, mul=sw[:, 0:1])
                    nc.scalar.mul(out=vsx, in_=vs_ps, mul=sw[:, 0:1])
                    nc.scalar.copy(out=acx, in_=ac_ps)
                    nc.scalar.copy(out=asx, in_=as_ps)
                    t1 = pwork.tile([NB, BLK], F32R, tag="t1")
                    t2 = pwork.tile([NB, BLK], F32R, tag="t2")
                    t3 = pwork.tile([NB, BLK], F32R, tag="t3")
                    t4 = pwork.tile([NB, BLK], F32R, tag="t4")
                    nc.gpsimd.tensor_mul(t1, vcx, acx)
                    nc.gpsimd.tensor_mul(t2, vsx, asx)
                    nc.gpsimd.tensor_mul(t3, vcx, asx)
                    nc.gpsimd.tensor_mul(t4, vsx, acx)
                    pr = pwork.tile([NB, BLK], F32R, tag="pr")
                    pi = pwork.tile([NB, BLK], F32R, tag="pi")
                    nc.gpsimd.tensor_add(pr, t1, t2)
                    nc.gpsimd.tensor_sub(pi, t3, t4)
                    corr = psc.tile([128, NSUB, 256], F32, tag="corr")
                    for s in range(NSUB):
                        sl = slice(s * 128, (s + 1) * 128)
                        nc.tensor.matmul(corr[:, s, :], pr[:, sl], ci, start=True, stop=False)
                        nc.tensor.matmul(corr[:, s, :], pi[:, sl], si, start=False, stop=True)
                    # sumsq: sum_k corr^2 = 4*sum_p(pr^2+pi^2) via ACT squares + tiny mm
                    sq1 = pwork.tile([NB, BLK], FP16, tag="sq1")
                    sq2 = pwork.tile([NB, BLK], FP16, tag="sq2")
                    nc.gpsimd.tensor_mul(sq1, pr, pr)
                    nc.gpsimd.tensor_mul(sq2, pi, pi)
                    ssq_ps = vc_ps[:, 8:12]          # dead region after muls
                    for s in range(NSUB):
                        sl = slice(s * 128, (s + 1) * 128)
                        nc.tensor.matmul(ssq_ps[:, s:s + 1], sq1[:, sl], ones,
                                         start=True, stop=False)
                        nc.tensor.matmul(ssq_ps[:, s:s + 1], sq2[:, sl], ones,
                                         start=False, stop=True)
                    nc.vector.tensor_copy(out=ssq_c[:, blk, :], in_=ssq_ps)
                    nc.vector.reduce_max(im8[:, 0:4], corr[:, :, 0:255], axis=AX.X)
                    nc.vector.tensor_copy(out=cmax[:, blk, :], in_=im8[:, 0:4])
                    idx8 = mxw.tile([128, 8], U32, tag="idx8")
                    nc.vector.max_index(idx8, im8, corr.rearrange("p a b -> p (a b)"))
                    nc.vector.tensor_copy(out=idxf[:, blk, :], in_=idx8[:, 0:4])
                    nc.vector.tensor_copy(out=s_c[:, blk, :], in_=corr[:, :, 255])
                    # norm tiny-mms for group `norm_g` land in this block's
                    # as_ps[:, 8:24] (dead after the stt muls consumed it)
                    if norm_g is not None:
                        cp = norm_g * 8
                        vsq = sqp.tile([128, 8, L], FP16, tag="vsq")
                        asq = sqp.tile([128, 8, L], FP16, tag="asq")
                        nc.scalar.activation(out=vsq, in_=TH[:, norm_g, 0],
                                             func=AF.Square, bias=zero_b)
                        nc.gpsimd.tensor_mul(asq, TH[:, norm_g, 1], TH[:, norm_g, 1])
                        for s in range(8):
                            nc.tensor.matmul(as_ps[:, 8 + s:9 + s], vsq[:, s, :], ones,
                                             start=True, stop=True)
                            nc.tensor.matmul(as_ps[:, 16 + s:17 + s], asq[:, s, :], ones,
                                             start=True, stop=True)
                        nc.vector.tensor_copy(out=nsq[:, 0, cp:cp + 8], in_=as_ps[:, 8:16])
                        nc.vector.tensor_copy(out=nsq[:, 1, cp:cp + 8], in_=as_ps[:, 16:24])

                nall = stats.tile([128, 2, NSUBALL], F32, tag="nall")

                def inv_part(lo, hi):
                    nc.vector.tensor_scalar_max(nsq[:, :, lo:hi], nsq[:, :, lo:hi], 1e-24)
                    nc.scalar.activation(out=nall[:, :, lo:hi], in_=nsq[:, :, lo:hi],
                                         func=AF.Sqrt, bias=zero_b)
                    nc.vector.reciprocal(out=inv_v[:, lo:hi], in_=nall[:, 0, lo:hi])
                    nc.vector.reciprocal(out=inv_a[:, lo:hi], in_=nall[:, 1, lo:hi])

                def sins_blk(blk):
                    c0 = blk * 4
                    a_sc = sqp.tile([128, NSUB, L], FP16, tag="a_sc")
                    v_sc = sqp.tile([128, NSUB, L], FP16, tag="v_sc")
                    d_t = sqp.tile([128, NSUB, L], FP16, tag="d_t")
                    for s in range(NSUB):
                        gs, ls = (c0 + s) // 8, (c0 + s) % 8
                        nc.gpsimd.tensor_scalar_mul(a_sc[:, s, :], SH[:, gs, 1, ls, :],
                                                    inv_a[:, c0 + s:c0 + s + 1])
                        nc.gpsimd.tensor_scalar_mul(v_sc[:, s, :], SH[:, gs, 0, ls, :],
                                                    inv_v[:, c0 + s:c0 + s + 1])
                    nc.gpsimd.tensor_sub(d_t, v_sc, a_sc)
                    sc2 = sqp.tile([128, 2, NSUB, L], FP16, tag="sc2")
                    nc.scalar.activation(out=sc2[:, 0], in_=d_t, func=AF.Sin, bias=zero_b)
                    nc.scalar.activation(out=sc2[:, 1], in_=d_t, func=AF.Sin, bias=halfpi_b)
                    msc = sqp.tile([128, 2, NSUB], F32, tag="msc")
                    nc.vector.reduce_sum(msc, sc2, axis=AX.X)
                    nc.vector.tensor_copy(out=ms[:, blk, :], in_=msc[:, 0])
                    nc.vector.tensor_copy(out=mc[:, blk, :], in_=msc[:, 1])

                # interleaved emission: dma prefetch, phase0(g), blocks of
                # g-1, early sins once the first inv half is ready
                phase0_dma(0)
                sins_sched = {5: [0, 1, 2], 6: [3, 4, 5], 7: [6, 7]}
                for g in range(NGRP):
                    if g + 1 < NGRP:
                        phase0_dma(g + 1)
                    phase0(g)
                    if g >= 1:
                        phase1(2 * (g - 1), None)
                        phase1(2 * (g - 1) + 1, g - 1)
                    if g == 4:
                        inv_part(0, 32)
                    for b in sins_sched.get(g, []):
                        sins_blk(b)
                phase1(NBLK - 2, None)
                phase1(NBLK - 1, NGRP - 1)
                inv_part(32, 64)
                for b in range(8, NBLK):
                    sins_blk(b)

            # phase 2: assemble features + MLP
            with tc.tile_pool(name="asm", bufs=1) as asm, \
                 tc.tile_pool(name="mlpw", bufs=4) as mlpw, \
                 tc.tile_pool(name="psm", bufs=2, space="PSUM") as psm:
                sh = [128, NBLK, 4]
                u = asm.tile(sh, F32, tag="u")
                u2 = asm.tile(sh, F32, tag="u2")
                tA = asm.tile(sh, F32, tag="tA")
                tB = asm.tile(sh, F32, tag="tB")
                tC = asm.tile(sh, F32, tag="tC")
                iv4 = inv_v.rearrange("p (a b) -> p a b", b=4)
                ia4 = inv_a.rearrange("p (a b) -> p a b", b=4)
                nc.vector.tensor_mul(u, iv4, ia4)
                nc.vector.tensor_mul(u2, u, u)
                uq = asm.tile(sh, F32, tag="uq")
                nc.vector.tensor_scalar_mul(uq, u, 1.0 / (1.0 + 1e-6))
                nc.vector.tensor_mul(s_all[:, :, :, 1], cmax, uq)
                nc.vector.tensor_scalar_mul(tA, s_c, 1.0 / 255.0)
                nc.vector.tensor_mul(tA, tA, u)
                nc.vector.tensor_mul(tB, tA, tA)
                nc.vector.tensor_scalar_mul(tC, ssq_c, 4.0 / 255.0)
                nc.vector.tensor_mul(tC, tC, u2)
                nc.vector.tensor_sub(tC, tC, tB)
                nc.vector.tensor_scalar_max(tC, tC, 0.0)
                nc.scalar.activation(out=s_all[:, :, :, 2], in_=tC, func=AF.Sqrt, bias=zero_b)
                nc.vector.tensor_scalar(out=s_all[:, :, :, 3], in0=u, scalar1=0.0,
                                        scalar2=1.0 / (1.0 + 1e-6), op0=ALU.mult,
                                        op1=ALU.add)
                for s in range(4):
                    off = float(s * 256 + 127)
                    nc.vector.tensor_scalar(out=tA[:, :, s], in0=idxf[:, :, s],
                                            scalar1=off, scalar2=0.1,
                                            op0=ALU.subtract, op1=ALU.mult)
                nc.vector.tensor_copy(out=s_all[:, :, :, 0], in_=tA)
                nc.scalar.activation(out=tB, in_=tA, func=AF.Abs, bias=zero_b, scale=10.0)
                nc.vector.tensor_scalar(out=tB, in0=tB, scalar1=1.0, scalar2=None,
                                        op0=ALU.add)
                nc.vector.reciprocal(out=tC, in_=tB)
                nc.vector.tensor_copy(out=s_all[:, :, :, 5], in_=tC)
                nc.vector.tensor_mul(tA, mc, mc)
                nc.vector.tensor_mul(tB, ms, ms)
                nc.vector.tensor_add(tC, tA, tB)
                nc.scalar.activation(out=s_all[:, :, :, 4], in_=tC, func=AF.Sqrt,
                                     bias=zero_b, scale=1.0 / (128.0 * 128.0))

                for g in range(NBLK // 4):
                    sT_ps = psm.tile([24, BLK], F32R, tag="sT_ps")
                    for t_ in range(4):
                        b = g * 4 + t_
                        nc.tensor.transpose(sT_ps[:, t_ * 128:(t_ + 1) * 128],
                                            s_all[:, b, :, :].rearrange("p a b -> p (a b)"),
                                            identr)
                    sT = mlpw.tile([24, BLK], F32R, tag="sT")
                    nc.scalar.copy(out=sT, in_=sT_ps)
                    h_ps = psm.tile([64, BLK], F32, tag="h_ps")
                    nc.tensor.matmul(h_ps, w1, sT, start=True, stop=True)
                    h_sb = mlpw.tile([64, BLK], F32R, tag="h_sb")
                    nc.scalar.activation(out=h_sb, in_=h_ps, func=AF.Relu, bias=b1)
                    o_ps = psm.tile([128, BLK], F32, tag="o_ps")
                    nc.tensor.matmul(o_ps, w2, h_sb, start=True, stop=True)
                    o_sb = mlpw.tile([128, BLK], F32R, tag="o_sb")
                    nc.scalar.activation(out=o_sb, in_=o_ps, func=AF.Identity, bias=b2)
                    oT_ps = psm.tile([128, BLK], F32R, tag="oT_ps")
                    for t_ in range(4):
                        nc.tensor.transpose(oT_ps[:, t_ * 128:(t_ + 1) * 128],
                                            o_sb[:, t_ * 128:(t_ + 1) * 128], identr)
                    oT = mlpw.tile([128, BLK], F32, tag="oT")
                    nc.vector.tensor_copy(out=oT, in_=oT_ps)
                    nc.sync.dma_start(
                        out=out[g * 4 * BLK:(g + 1) * 4 * BLK, :].rearrange(
                            "(blk sub p) f -> p blk sub f", p=128, sub=4),
                        in_=oT.rearrange("p (blk sub f) -> p blk sub f", sub=4, f=32))
    nc.compile()
    return nc


def make_in_map(video, audio, W1, b1, W2, b2):
    c = _consts()
    W1x4, W2x4, b1x4, b2x4 = _mlp_consts(
        np.asarray(W1, np.float32), np.asarray(b1, np.float32),
        np.asarray(W2, np.float32), np.asarray(b2, np.float32))
    return {
        "video": video, "audio": audio,
        "wch": c["wch"], "wcl": c["wcl"], "wsh": c["wsh"], "wsl": c["wsl"],
        "ci": c["ci"], "si": c["si"], "sw": c["sw"], "ident": c["ident"],
        "ones": c["ones"],
        "w1x4": W1x4, "w2x4": W2x4, "b1x4": b1x4, "b2x4": b2x4,
    }


def kernel(video_features, audio_features, W1, b1, W2, b2):
    video_features = np.ascontiguousarray(np.asarray(video_features, dtype=np.float32))
    audio_features = np.ascontiguousarray(np.asarray(audio_features, dtype=np.float32))
    if "nc" not in _CACHED:
        _CACHED["nc"] = build_nc()
    nc = _CACHED["nc"]

    in_maps = []
    for i in range(NCORES):
        sl = slice(i * SPC, (i + 1) * SPC)
        in_maps.append(make_in_map(video_features[sl], audio_features[sl],
                                   W1, b1, W2, b2))
    res = run_bass_kernel_spmd(nc, in_maps, list(range(NCORES)))
    out = np.concatenate([res.results[i]["out"] for i in range(NCORES)], axis=0)
    return out.astype(np.float32)


if __name__ == "__main__":
    rng = np.random.default_rng(0)
    inputs = dict(
        video_features=rng.standard_normal((B_FULL, L)).astype(np.float32),
        audio_features=rng.standard_normal((B_FULL, L)).astype(np.float32),
        W1=(rng.standard_normal((6, 16)) * 0.3).astype(np.float32),
        b1=(rng.standard_normal((16,)) * 0.1).astype(np.float32),
        W2=(rng.standard_normal((16, 32)) * 0.2).astype(np.float32),
        b2=(rng.standard_normal((32,)) * 0.1).astype(np.float32),
    )
    out = kernel(**inputs)
    print("out", out.shape, out.dtype, np.abs(out).mean())
